# revision 18
# baseline (speedup 1.0000x reference)
"""Trainium2 Bass kernel for nn_Block_89859305767239 (dense transformer block).

Reference computation (B=2, T=2048, D=768, H=12, HS=64, HID=3072):
    xn = LN(x);  q,k,v per-head linears of xn
    wei[t,s] = (k[t] . q[s]) * D**-0.5, causal tril mask, softmax over s
    o = attn @ v, concat heads, proj (Wp) + residual
    x2 = x + o;  y = x2 + relu(LN(x2) @ W1.T + b1) @ W2.T + b2
    returns (y, attn)

Sharding (8 cores): the 24 (b, h) pairs are split 3-per-core (core c gets
pairs 3c..3c+2, all within batch b=c//4).  Each core computes LN + its
heads' q/k/v, full causal attention rows for its pairs (writing its slice
of the attn output), then a partial attention-projection over its 192
channels for all 2048 tokens of its batch.  A 4-core ReduceScatter
(groups [[0..3],[4..7]]) sums the partials and hands each core its own
512-token slice, on which it runs residual + LN2 + FFN and writes its
slice of y.  Matmuls run in bf16 (fp32 accumulation in PSUM); softmax,
layernorm and residuals are fp32.
"""

import sys

sys.path.insert(0, "/opt/trn_rl_repo")

from contextlib import ExitStack

import ml_dtypes
import numpy as np

import concourse.bass as bass
import concourse.bacc as bacc
import concourse.tile as tile
from concourse import mybir
from concourse.bass import ts
from concourse.bass_utils import run_bass_kernel_spmd
from concourse.masks import make_identity

F32 = mybir.dt.float32
BF16 = mybir.dt.bfloat16
BF = ml_dtypes.bfloat16
AF = mybir.ActivationFunctionType
ALU = mybir.AluOpType
AX = mybir.AxisListType

T, D, H, HS, HID = 2048, 768, 12, 64, 3072
P = 128
NCORES = 8
PAIRS = 3            # (b, h) pairs per core
TSL = 512            # token slice per core (phase 2)
SCALE = float(D) ** -0.5
EPS = 1e-5
VCORR = float(D) / float(D - 1)   # unbiased-variance (ddof=1) correction
NT = T // P          # 16 row tiles
NCC = D // P         # 6 contraction chunks of 128
NHT = HID // P       # 24 hidden chunks


def _bcast(ap, parts):
    """Partition-broadcast view of a [1, N] access pattern."""
    return bass.AP(tensor=ap.tensor, offset=ap.offset, ap=[[0, parts]] + list(ap.ap[1:]))


def _layernorm(nc, spool, x_ap, xn_out, eps_ap):
    """LN over the free dim (768) of a [128, 768] fp32 tile -> bf16 out."""
    stats = spool.tile([P, 3, 6], F32, tag="lnstats")
    for sg in range(3):
        nc.vector.bn_stats(out=stats[:, sg, :], in_=x_ap[:, sg * 256:(sg + 1) * 256])
    mv = spool.tile([P, 2], F32, tag="lnmv")
    nc.vector.bn_aggr(out=mv, in_=stats)
    rstd = spool.tile([P, 1], F32, tag="lnrstd")
    # rstd = 1/sqrt(var * D/(D-1) + eps)
    nc.scalar.activation(out=rstd, in_=mv[:, 1:2], func=AF.Sqrt, bias=eps_ap, scale=VCORR)
    nc.vector.reciprocal(out=rstd, in_=rstd)
    nc.vector.tensor_scalar(
        out=xn_out, in0=x_ap, scalar1=mv[:, 0:1], scalar2=rstd,
        op0=ALU.subtract, op1=ALU.mult,
    )


def _build_program():
    nc = bacc.Bacc("TRN2", target_bir_lowering=False, debug=False)

    xb = nc.declare_dram_parameter("xb", [T, D], F32, isOutput=False)
    xsp = nc.declare_dram_parameter("xsp", [TSL, D], F32, isOutput=False)
    wq = nc.declare_dram_parameter("wq", [PAIRS, P, NCC * HS], BF16, isOutput=False)
    wk = nc.declare_dram_parameter("wk", [PAIRS, P, NCC * HS], BF16, isOutput=False)
    wv = nc.declare_dram_parameter("wv", [PAIRS, P, NCC * HS], BF16, isOutput=False)
    bq = nc.declare_dram_parameter("bq", [HS, PAIRS], F32, isOutput=False)
    bk = nc.declare_dram_parameter("bk", [HS, PAIRS], F32, isOutput=False)
    bv = nc.declare_dram_parameter("bv", [1, PAIRS * HS], F32, isOutput=False)
    wp = nc.declare_dram_parameter("wp", [PAIRS, HS, D], BF16, isOutput=False)
    w1 = nc.declare_dram_parameter("w1", [NHT, P, D], BF16, isOutput=False)
    b1 = nc.declare_dram_parameter("b1", [P, NHT], F32, isOutput=False)
    w2 = nc.declare_dram_parameter("w2", [NHT, P, D], BF16, isOutput=False)
    b2 = nc.declare_dram_parameter("b2", [1, D], F32, isOutput=False)

    attn = nc.declare_dram_parameter("attn", [PAIRS, T, T], F32, isOutput=True)
    y = nc.declare_dram_parameter("y", [TSL, D], F32, isOutput=True)

    with tile.TileContext(nc) as tc, ExitStack() as ctx:
        consts = ctx.enter_context(tc.tile_pool(name="consts", bufs=1))
        wpool = ctx.enter_context(tc.tile_pool(name="wpool", bufs=1))
        spool = ctx.enter_context(tc.tile_pool(name="spool", bufs=6))
        xstream = ctx.enter_context(tc.tile_pool(name="xstream", bufs=3))
        epool = ctx.enter_context(tc.tile_pool(name="epool", bufs=2))
        etp = ctx.enter_context(tc.tile_pool(name="etp", bufs=18))
        opool = ctx.enter_context(tc.tile_pool(name="opool", bufs=3))
        ppool = ctx.enter_context(tc.tile_pool(name="ppool", bufs=3))
        wstream = ctx.enter_context(tc.tile_pool(name="wstream", bufs=3))
        ypool = ctx.enter_context(tc.tile_pool(name="ypool", bufs=2))
        dram = ctx.enter_context(tc.tile_pool(name="dram", bufs=1, space="DRAM"))

        ps_main = ctx.enter_context(tc.tile_pool(name="ps_main", bufs=4, space="PSUM"))
        ps_sm = ctx.enter_context(tc.tile_pool(name="ps_sm", bufs=2, space="PSUM"))
        ps_t = ctx.enter_context(tc.tile_pool(name="ps_t", bufs=2, space="PSUM"))

        # ---- constants ----
        ident = consts.tile([P, P], BF16)
        make_identity(nc, ident)
        ztile = consts.tile([P, T], F32)
        nc.vector.memset(ztile, 0.0)
        ones_row = consts.tile([1, P], F32)
        nc.vector.memset(ones_row, 1.0)
        eps_t = consts.tile([P, 1], F32)
        nc.vector.memset(eps_t, EPS)
        # tril[x,y] = 1 if x>=y else 0 (keep s<=t for E rows);  triu = tril.T
        tril_m = consts.tile([P, P], F32)
        nc.gpsimd.memset(tril_m, 1.0)
        nc.gpsimd.affine_select(out=tril_m, in_=tril_m, compare_op=ALU.is_ge,
                                fill=0.0, base=0, channel_multiplier=1,
                                pattern=[[-1, P]])
        triu_m = consts.tile([P, P], BF16)
        nc.gpsimd.memset(triu_m, 1.0)
        nc.gpsimd.affine_select(out=triu_m, in_=triu_m, compare_op=ALU.is_ge,
                                fill=0.0, base=0, channel_multiplier=-1,
                                pattern=[[1, P]])
        b2bc = consts.tile([P, D], F32)
        nc.sync.dma_start(out=b2bc, in_=_bcast(b2[0:1, :], P))
        b1_sb = consts.tile([P, NHT], F32)
        nc.sync.dma_start(out=b1_sb, in_=b1[:, :])
        bq_sb = consts.tile([HS, PAIRS], F32)
        nc.sync.dma_start(out=bq_sb, in_=bq[:, :])
        bk_sb = consts.tile([HS, PAIRS], F32)
        nc.sync.dma_start(out=bk_sb, in_=bk[:, :])
        bv_sb = consts.tile([1, PAIRS * HS], F32)
        nc.sync.dma_start(out=bv_sb, in_=bv[:, :])

        # ---- resident weights ----
        wq_sb = wpool.tile([P, PAIRS, NCC * HS], BF16)
        wk_sb = wpool.tile([P, PAIRS, NCC * HS], BF16)
        wv_sb = wpool.tile([P, PAIRS, NCC * HS], BF16)
        for hh in range(PAIRS):
            nc.sync.dma_start(out=wq_sb[:, hh, :], in_=wq[hh])
            nc.sync.dma_start(out=wk_sb[:, hh, :], in_=wk[hh])
            nc.sync.dma_start(out=wv_sb[:, hh, :], in_=wv[hh])
        wp_sb = wpool.tile([HS, PAIRS, D], BF16)
        for hh in range(PAIRS):
            nc.sync.dma_start(out=wp_sb[:, hh, :], in_=wp[hh])

        # ---- internal DRAM for the collective ----
        rs_in = dram.tile([4, TSL, D], BF16)
        rs_out = dram.tile([TSL, D], BF16)

        # ================= Phase A: LN1 + transpose =================
        with tc.tile_pool(name="biga", bufs=1) as biga:
            xnT = biga.tile([P, NCC, T], BF16)     # [c within chunk, chunk, t]
            for tt in range(NT):
                xt = xstream.tile([P, D], F32, tag="xt")
                nc.gpsimd.dma_start(out=xt, in_=xb[ts(tt, P), :])
                xn = xstream.tile([P, D], BF16, tag="xn")
                _layernorm(nc, spool, xt, xn, eps_t)
                for cc in range(NCC):
                    tp = ps_t.tile([P, P], BF16, tag="pst")
                    nc.tensor.transpose(out=tp, in_=xn[:, ts(cc, P)], identity=ident)
                    nc.any.tensor_copy(out=xnT[:, cc, ts(tt, P)], in_=tp)

            # ================= Phase B: Q/K/V =================
            qT = biga.tile([HS, PAIRS, T], BF16)    # [d, pair, t]
            kT = biga.tile([HS, PAIRS, T], BF16)
            vS = biga.tile([P, PAIRS, NT, HS], BF16)  # [s within chunk, pair, s chunk, d]
            for hh in range(PAIRS):
                for tq in range(T // 512):
                    psq = ps_main.tile([HS, 512], F32, tag="ps")
                    psk = ps_main.tile([HS, 512], F32, tag="ps")
                    for cc in range(NCC):
                        nc.tensor.matmul(
                            psq, lhsT=wq_sb[:, hh, ts(cc, HS)],
                            rhs=xnT[:, cc, ts(tq, 512)],
                            start=(cc == 0), stop=(cc == NCC - 1))
                    nc.scalar.activation(out=qT[:, hh, ts(tq, 512)], in_=psq,
                                         func=AF.Identity, bias=bq_sb[:, hh:hh + 1])
                    for cc in range(NCC):
                        nc.tensor.matmul(
                            psk, lhsT=wk_sb[:, hh, ts(cc, HS)],
                            rhs=xnT[:, cc, ts(tq, 512)],
                            start=(cc == 0), stop=(cc == NCC - 1))
                    nc.scalar.activation(out=kT[:, hh, ts(tq, 512)], in_=psk,
                                         func=AF.Identity, bias=bk_sb[:, hh:hh + 1])
                for st in range(NT):
                    psv = ps_sm.tile([P, HS], F32, tag="pacc")
                    for cc in range(NCC):
                        nc.tensor.matmul(
                            psv, lhsT=xnT[:, cc, ts(st, P)],
                            rhs=wv_sb[:, hh, ts(cc, HS)],
                            start=(cc == 0), stop=False)
                    # += bv (broadcast over tokens via rank-1 matmul)
                    nc.tensor.matmul(psv, lhsT=ones_row, rhs=bv_sb[:, ts(hh, HS)],
                                     start=False, stop=True)
                    nc.any.tensor_copy(out=vS[:, hh, st, :], in_=psv)

            # ================= Phase C: attention =================
            oT = [biga.tile([HS, T], BF16, name=f"oT{hh}", tag=f"oT{hh}") for hh in range(PAIRS)]
            for hh in range(PAIRS):
                recips = spool.tile([P, NT], F32, tag="recips")
                for i in range(T // 512):      # t quads (for E^T / AV)
                    ets = []
                    for j in range(4 * i + 4):  # s chunks of 128
                        pse = ps_main.tile([P, 512], F32, tag="ps")
                        nc.tensor.matmul(pse, lhsT=qT[:, hh, ts(j, P)],
                                         rhs=kT[:, hh, ts(i, 512)],
                                         start=True, stop=True)
                        et = etp.tile([P, 512], BF16, tag="et")
                        nc.scalar.activation(out=et, in_=pse, func=AF.Exp, scale=SCALE)
                        if j >= 4 * i:
                            u = j - 4 * i
                            if u > 0:
                                nc.gpsimd.memset(et[:, 0:u * P], 0.0)
                            # diag block: keep t >= s
                            nc.vector.tensor_mul(out=et[:, ts(u, P)],
                                                 in0=et[:, ts(u, P)], in1=triu_m)
                        ets.append(et)
                    for u in range(4):
                        r = 4 * i + u
                        L = P * (r + 1)
                        E = epool.tile([P, T], F32, tag="E")
                        sums = spool.tile([P, 8], F32, tag="sums")
                        nq = r // 4 + 1
                        for qd in range(nq):
                            W = 512 if qd < r // 4 else P * (r % 4 + 1)
                            pss = ps_main.tile([P, 512], F32, tag="ps")
                            nc.tensor.matmul(pss[:, :W], lhsT=kT[:, hh, ts(r, P)],
                                             rhs=qT[:, hh, qd * 512:qd * 512 + W],
                                             start=True, stop=True)
                            if qd < r // 4:
                                nc.scalar.activation(out=E[:, ts(qd, 512)], in_=pss,
                                                     func=AF.Exp, scale=SCALE,
                                                     accum_out=sums[:, qd:qd + 1])
                            else:
                                off = qd * 512
                                if W > P:
                                    nc.scalar.activation(
                                        out=E[:, off:off + W - P], in_=pss[:, :W - P],
                                        func=AF.Exp, scale=SCALE,
                                        accum_out=sums[:, qd:qd + 1])
                                else:
                                    nc.vector.memset(sums[:, qd:qd + 1], 0.0)
                                ds = off + W - P   # == P*r
                                nc.scalar.activation(out=E[:, ds:ds + P],
                                                     in_=pss[:, W - P:W],
                                                     func=AF.Exp, scale=SCALE)
                                nc.vector.tensor_mul(out=E[:, ds:ds + P],
                                                     in0=E[:, ds:ds + P], in1=tril_m)
                                nc.vector.reduce_sum(out=sums[:, nq:nq + 1],
                                                     in_=E[:, ds:ds + P], axis=AX.X)
                        tot = spool.tile([P, 1], F32, tag="tot")
                        nc.vector.reduce_sum(out=tot, in_=sums[:, :nq + 1], axis=AX.X)
                        nc.vector.reciprocal(out=recips[:, r:r + 1], in_=tot)
                        Eo = epool.tile([P, T], F32, tag="Eo")
                        nc.scalar.activation(out=Eo[:, :L], in_=E[:, :L],
                                             func=AF.Copy,
                                             scale=recips[:, r:r + 1])
                        nc.sync.dma_start(out=attn[hh, ts(r, P), 0:L], in_=Eo[:, :L])
                        if L < T:
                            nc.gpsimd.dma_start(out=attn[hh, ts(r, P), L:T],
                                                in_=ztile[:, 0:T - L])
                        # o row-tile: accumulate over valid s chunks
                        pso = ps_sm.tile([P, HS], F32, tag="pacc")
                        for j in range(r + 1):
                            nc.tensor.matmul(pso, lhsT=ets[j][:, ts(u, P)],
                                             rhs=vS[:, hh, j, :],
                                             start=(j == 0), stop=(j == r))
                        osb = opool.tile([P, HS], BF16, tag="osb")
                        nc.vector.tensor_scalar_mul(out=osb, in0=pso,
                                                    scalar1=recips[:, r:r + 1])
                        tp = ps_t.tile([P, P], BF16, tag="pst")
                        nc.tensor.transpose(out=tp[:HS, :], in_=osb, identity=ident)
                        nc.any.tensor_copy(out=oT[hh][:, ts(r, P)], in_=tp[:HS, :])

            # ================= Phase D: partial projection + RS =================
            for tt in range(NT):
                pp = ppool.tile([P, D], BF16, tag="pp")
                for half in range(2):
                    psp = ps_main.tile([P, 384], F32, tag="ps")
                    for hh in range(PAIRS):
                        nc.tensor.matmul(psp, lhsT=oT[hh][:, ts(tt, P)],
                                         rhs=wp_sb[:, hh, ts(half, 384)],
                                         start=(hh == 0), stop=(hh == PAIRS - 1))
                    nc.scalar.activation(out=pp[:, ts(half, 384)], in_=psp, func=AF.Copy)
                nc.sync.dma_start(
                    out=rs_in[tt // 4, (tt % 4) * P:(tt % 4 + 1) * P, :], in_=pp)

        nc.gpsimd.collective_compute(
            "ReduceScatter", ALU.add,
            replica_groups=[[0, 1, 2, 3], [4, 5, 6, 7]],
            ins=[rs_in.opt()], outs=[rs_out.opt()])

        # ================= Phase E: residual + LN2 + FFN =================
        with tc.tile_pool(name="bigb", bufs=1) as bigb:
            x2 = bigb.tile([P, 4, D], F32)
            xn2T = bigb.tile([P, NCC, TSL], BF16)
            for t2 in range(4):
                rso = ypool.tile([P, D], BF16, tag="rso")
                nc.gpsimd.dma_start(out=rso, in_=rs_out[ts(t2, P), :])
                xst = xstream.tile([P, D], F32, tag="xt")
                nc.gpsimd.dma_start(out=xst, in_=xsp[ts(t2, P), :])
                nc.vector.tensor_add(out=x2[:, t2, :], in0=xst, in1=rso)
                xn2 = xstream.tile([P, D], BF16, tag="xn")
                _layernorm(nc, spool, x2[:, t2, :], xn2, eps_t)
                for cc in range(NCC):
                    tp = ps_t.tile([P, P], BF16, tag="pst")
                    nc.tensor.transpose(out=tp, in_=xn2[:, ts(cc, P)], identity=ident)
                    nc.any.tensor_copy(out=xn2T[:, cc, ts(t2, P)], in_=tp)

            h1T = bigb.tile([P, NHT, TSL], BF16)
            w2_sb = bigb.tile([P, NHT, D], BF16)
            for ht in range(NHT):
                nc.gpsimd.dma_start(out=w2_sb[:, ht, :], in_=w2[ht])
                w1t = wstream.tile([P, D], BF16, tag="w1t")
                nc.gpsimd.dma_start(out=w1t, in_=w1[ht])
                psh = ps_main.tile([P, TSL], F32, tag="ps")
                for cc in range(NCC):
                    nc.tensor.matmul(psh, lhsT=w1t[:, ts(cc, P)], rhs=xn2T[:, cc, :],
                                     start=(cc == 0), stop=(cc == NCC - 1))
                nc.scalar.activation(out=h1T[:, ht, :], in_=psh, func=AF.Relu,
                                     bias=b1_sb[:, ht:ht + 1])
            for t2 in range(4):
                ysb = ypool.tile([P, D], F32, tag="ysb")
                for half in range(2):
                    ps2 = ps_main.tile([P, 384], F32, tag="ps")
                    for ht in range(NHT):
                        nc.tensor.matmul(ps2, lhsT=h1T[:, ht, ts(t2, P)],
                                         rhs=w2_sb[:, ht, ts(half, 384)],
                                         start=(ht == 0), stop=(ht == NHT - 1))
                    nc.vector.tensor_add(out=ysb[:, ts(half, 384)], in0=ps2,
                                         in1=x2[:, t2, ts(half, 384)])
                nc.vector.tensor_add(out=ysb, in0=ysb, in1=b2bc)
                nc.sync.dma_start(out=y[ts(t2, P), :], in_=ysb)

    nc.compile()
    return nc


def _prep_in_maps(inputs):
    x = np.asarray(inputs["x"], np.float32)
    Wq = np.asarray(inputs["Wq"], np.float32)
    Wk = np.asarray(inputs["Wk"], np.float32)
    Wv = np.asarray(inputs["Wv"], np.float32)
    bq = np.asarray(inputs["bq"], np.float32)
    bk = np.asarray(inputs["bk"], np.float32)
    bv = np.asarray(inputs["bv"], np.float32)
    Wp = np.asarray(inputs["Wp"], np.float32)
    bp = np.asarray(inputs["bp"], np.float32)
    W1 = np.asarray(inputs["W1"], np.float32)
    b1 = np.asarray(inputs["b1"], np.float32)
    W2 = np.asarray(inputs["W2"], np.float32)
    b2 = np.asarray(inputs["b2"], np.float32)

    def wqkv_slot(Wm, h):
        # [128, 6*64]: [p, cc*64+j] = Wm[h][j, cc*128+p]
        return np.ascontiguousarray(
            Wm[h].reshape(HS, NCC, P).transpose(2, 1, 0).reshape(P, NCC * HS)
        ).astype(BF)

    w1_host = np.ascontiguousarray(
        W1.reshape(NHT, P, NCC, P).transpose(0, 3, 2, 1).reshape(NHT, P, D)
    ).astype(BF)
    b1_host = np.ascontiguousarray(b1.reshape(NHT, P).T).astype(np.float32)
    w2_host = np.ascontiguousarray(
        W2.reshape(D, NHT, P).transpose(1, 2, 0)
    ).astype(BF)
    b2_host = b2.reshape(1, D).astype(np.float32)

    in_maps = []
    for c in range(NCORES):
        b, m = c // 4, c % 4
        hs = [3 * c % 12 + i for i in range(PAIRS)]
        im = {
            "xb": np.ascontiguousarray(x[b]),
            "xsp": np.ascontiguousarray(x[b, 512 * m:512 * (m + 1)] + bp[None, :]),
            "wq": np.stack([wqkv_slot(Wq, h) for h in hs]),
            "wk": np.stack([wqkv_slot(Wk, h) for h in hs]),
            "wv": np.stack([wqkv_slot(Wv, h) for h in hs]),
            "bq": np.ascontiguousarray(np.stack([bq[h] for h in hs], axis=1)),
            "bk": np.ascontiguousarray(np.stack([bk[h] for h in hs], axis=1)),
            "bv": np.concatenate([bv[h] for h in hs]).reshape(1, PAIRS * HS),
            "wp": np.stack(
                [np.ascontiguousarray(Wp[:, h * HS:(h + 1) * HS].T).astype(BF)
                 for h in hs]),
            "w1": w1_host,
            "b1": b1_host,
            "w2": w2_host,
            "b2": b2_host,
        }
        in_maps.append(im)
    return in_maps


_CACHED = {}


def run(inputs, trace=False):
    if "nc" not in _CACHED:
        _CACHED["nc"] = _build_program()
    nc = _CACHED["nc"]
    in_maps = _prep_in_maps(inputs)
    res = run_bass_kernel_spmd(nc, in_maps, list(range(NCORES)), trace=trace)
    ys = np.stack([np.asarray(res.results[c]["y"]) for c in range(NCORES)])
    attns = np.stack([np.asarray(res.results[c]["attn"]) for c in range(NCORES)])
    y_full = ys.reshape(2, T, D)
    attn_full = attns.reshape(2, H, T, T)
    return (y_full, attn_full), res


def kernel(**inputs):
    out, _ = run(inputs, trace=False)
    return out


# revision 19
# speedup vs baseline: 85.9634x; 85.9634x over previous
"""Trainium2 Bass kernel for nn_Block_89859305767239 (dense transformer block).

Reference computation (B=2, T=2048, D=768, H=12, HS=64, HID=3072):
    xn = LN(x);  q,k,v per-head linears of xn
    wei[t,s] = (k[t] . q[s]) * D**-0.5, causal tril mask, softmax over s
    o = attn @ v, concat heads, proj (Wp) + residual
    x2 = x + o;  y = x2 + relu(LN(x2) @ W1.T + b1) @ W2.T + b2
    returns (y, attn)

Sharding (8 cores): the 24 (b, h) pairs are split 3-per-core (core c gets
pairs 3c..3c+2, all within batch b=c//4).  Each core computes LN + its
heads' q/k/v, full causal attention rows for its pairs (writing its slice
of the attn output), then a partial attention-projection over its 192
channels for all 2048 tokens of its batch.  A 4-core ReduceScatter
(groups [[0..3],[4..7]]) sums the partials and hands each core its own
512-token slice, on which it runs residual + LN2 + FFN and writes its
slice of y.  Matmuls run in bf16 (fp32 accumulation in PSUM); softmax,
layernorm and residuals are fp32.
"""

import sys

sys.path.insert(0, "/opt/trn_rl_repo")

from contextlib import ExitStack

import ml_dtypes
import numpy as np

import concourse.bass as bass
import concourse.bacc as bacc
import concourse.tile as tile
from concourse import mybir
from concourse.bass import ts
from concourse.bass_utils import run_bass_kernel_spmd
from concourse.masks import make_identity

F32 = mybir.dt.float32
BF16 = mybir.dt.bfloat16
BF = ml_dtypes.bfloat16
AF = mybir.ActivationFunctionType
ALU = mybir.AluOpType
AX = mybir.AxisListType

T, D, H, HS, HID = 2048, 768, 12, 64, 3072
P = 128
NCORES = 8
PAIRS = 3            # (b, h) pairs per core
TSL = 512            # token slice per core (phase 2)
SCALE = float(D) ** -0.5
EPS = 1e-5
VCORR = float(D) / float(D - 1)   # unbiased-variance (ddof=1) correction
NT = T // P          # 16 row tiles
NCC = D // P         # 6 contraction chunks of 128
NHT = HID // P       # 24 hidden chunks


def _bcast(ap, parts):
    """Partition-broadcast view of a [1, N] access pattern."""
    return bass.AP(tensor=ap.tensor, offset=ap.offset, ap=[[0, parts]] + list(ap.ap[1:]))


def _layernorm(nc, spool, x_ap, xn_out, eps_ap):
    """LN over the free dim (768) of a [128, 768] fp32 tile -> bf16 out."""
    stats = spool.tile([P, 3, 6], F32, tag="lnstats")
    for sg in range(3):
        nc.vector.bn_stats(out=stats[:, sg, :], in_=x_ap[:, sg * 256:(sg + 1) * 256])
    mv = spool.tile([P, 2], F32, tag="lnmv")
    nc.vector.bn_aggr(out=mv, in_=stats)
    rstd = spool.tile([P, 1], F32, tag="lnrstd")
    # rstd = 1/sqrt(var * D/(D-1) + eps)
    nc.scalar.activation(out=rstd, in_=mv[:, 1:2], func=AF.Sqrt, bias=eps_ap, scale=VCORR)
    nc.vector.reciprocal(out=rstd, in_=rstd)
    nc.vector.tensor_scalar(
        out=xn_out, in0=x_ap, scalar1=mv[:, 0:1], scalar2=rstd,
        op0=ALU.subtract, op1=ALU.mult,
    )


def _build_program():
    nc = bacc.Bacc("TRN2", target_bir_lowering=False, debug=False)

    xb = nc.declare_dram_parameter("xb", [T, D], F32, isOutput=False)
    xsp = nc.declare_dram_parameter("xsp", [TSL, D], F32, isOutput=False)
    wq = nc.declare_dram_parameter("wq", [PAIRS, P, NCC * HS], BF16, isOutput=False)
    wk = nc.declare_dram_parameter("wk", [PAIRS, P, NCC * HS], BF16, isOutput=False)
    wv = nc.declare_dram_parameter("wv", [PAIRS, P, NCC * HS], BF16, isOutput=False)
    bq = nc.declare_dram_parameter("bq", [HS, PAIRS], F32, isOutput=False)
    bk = nc.declare_dram_parameter("bk", [HS, PAIRS], F32, isOutput=False)
    bv = nc.declare_dram_parameter("bv", [1, PAIRS * HS], F32, isOutput=False)
    wp = nc.declare_dram_parameter("wp", [PAIRS, HS, D], BF16, isOutput=False)
    w1 = nc.declare_dram_parameter("w1", [NHT, P, D], BF16, isOutput=False)
    b1 = nc.declare_dram_parameter("b1", [P, NHT], F32, isOutput=False)
    w2 = nc.declare_dram_parameter("w2", [NHT, P, D], BF16, isOutput=False)
    b2 = nc.declare_dram_parameter("b2", [1, D], F32, isOutput=False)

    attn = nc.declare_dram_parameter("attn", [PAIRS, T, T], F32, isOutput=True)
    y = nc.declare_dram_parameter("y", [TSL, D], F32, isOutput=True)

    with tile.TileContext(nc) as tc, ExitStack() as ctx:
        consts = ctx.enter_context(tc.tile_pool(name="consts", bufs=1))
        wpool = ctx.enter_context(tc.tile_pool(name="wpool", bufs=1))
        spool = ctx.enter_context(tc.tile_pool(name="spool", bufs=6))
        xstream = ctx.enter_context(tc.tile_pool(name="xstream", bufs=3))
        epool = ctx.enter_context(tc.tile_pool(name="epool", bufs=3))
        etp = ctx.enter_context(tc.tile_pool(name="etp", bufs=18))
        opool = ctx.enter_context(tc.tile_pool(name="opool", bufs=3))
        ppool = ctx.enter_context(tc.tile_pool(name="ppool", bufs=3))
        wstream = ctx.enter_context(tc.tile_pool(name="wstream", bufs=3))
        ypool = ctx.enter_context(tc.tile_pool(name="ypool", bufs=2))
        dram = ctx.enter_context(tc.tile_pool(name="dram", bufs=1, space="DRAM"))

        ps_main = ctx.enter_context(tc.tile_pool(name="ps_main", bufs=4, space="PSUM"))
        ps_sm = ctx.enter_context(tc.tile_pool(name="ps_sm", bufs=2, space="PSUM"))
        ps_t = ctx.enter_context(tc.tile_pool(name="ps_t", bufs=2, space="PSUM"))

        # ---- constants ----
        ident = consts.tile([P, P], BF16)
        make_identity(nc, ident)
        ones_row = consts.tile([1, P], F32)
        nc.vector.memset(ones_row, 1.0)
        eps_t = consts.tile([P, 1], F32)
        nc.vector.memset(eps_t, EPS)
        # tril[x,y] = 1 if x>=y else 0 (keep s<=t for E rows);  triu = tril.T
        tril_m = consts.tile([P, P], F32)
        nc.gpsimd.memset(tril_m, 1.0)
        nc.gpsimd.affine_select(out=tril_m, in_=tril_m, compare_op=ALU.is_ge,
                                fill=0.0, base=0, channel_multiplier=1,
                                pattern=[[-1, P]])
        triu_m = consts.tile([P, P], BF16)
        nc.gpsimd.memset(triu_m, 1.0)
        nc.gpsimd.affine_select(out=triu_m, in_=triu_m, compare_op=ALU.is_ge,
                                fill=0.0, base=0, channel_multiplier=-1,
                                pattern=[[1, P]])
        b2bc = consts.tile([P, D], F32)
        nc.sync.dma_start(out=b2bc, in_=_bcast(b2[0:1, :], P))
        b1_sb = consts.tile([P, NHT], F32)
        nc.sync.dma_start(out=b1_sb, in_=b1[:, :])
        bq_sb = consts.tile([HS, PAIRS], F32)
        nc.sync.dma_start(out=bq_sb, in_=bq[:, :])
        bk_sb = consts.tile([HS, PAIRS], F32)
        nc.sync.dma_start(out=bk_sb, in_=bk[:, :])
        bv_sb = consts.tile([1, PAIRS * HS], F32)
        nc.sync.dma_start(out=bv_sb, in_=bv[:, :])

        # ---- resident weights ----
        wq_sb = wpool.tile([P, PAIRS, NCC * HS], BF16)
        wk_sb = wpool.tile([P, PAIRS, NCC * HS], BF16)
        wv_sb = wpool.tile([P, PAIRS, NCC * HS], BF16)
        for hh in range(PAIRS):
            nc.sync.dma_start(out=wq_sb[:, hh, :], in_=wq[hh])
            nc.sync.dma_start(out=wk_sb[:, hh, :], in_=wk[hh])
            nc.sync.dma_start(out=wv_sb[:, hh, :], in_=wv[hh])
        wp_sb = wpool.tile([HS, PAIRS, D], BF16)
        for hh in range(PAIRS):
            nc.sync.dma_start(out=wp_sb[:, hh, :], in_=wp[hh])

        # ---- internal DRAM for the collective ----
        rs_in = dram.tile([4, TSL, D], BF16)
        rs_out = dram.tile([TSL, D], BF16)

        # ================= Phase A: LN1 + transpose =================
        with tc.tile_pool(name="biga", bufs=1) as biga:
            xnT = biga.tile([P, NCC, T], BF16)     # [c within chunk, chunk, t]
            for tt in range(NT):
                xt = xstream.tile([P, D], F32, tag="xt")
                nc.gpsimd.dma_start(out=xt, in_=xb[ts(tt, P), :])
                xn = xstream.tile([P, D], BF16, tag="xn")
                _layernorm(nc, spool, xt, xn, eps_t)
                for cc in range(NCC):
                    tp = ps_t.tile([P, P], BF16, tag="pst")
                    nc.tensor.transpose(out=tp, in_=xn[:, ts(cc, P)], identity=ident)
                    nc.any.tensor_copy(out=xnT[:, cc, ts(tt, P)], in_=tp)

            # ================= Phase B: Q/K/V =================
            qT = biga.tile([HS, PAIRS, T], BF16)    # [d, pair, t]
            kT = biga.tile([HS, PAIRS, T], BF16)
            vS = biga.tile([P, PAIRS, NT, HS], BF16)  # [s within chunk, pair, s chunk, d]
            for hh in range(PAIRS):
                for tq in range(T // 512):
                    psq = ps_main.tile([HS, 512], F32, tag="ps")
                    psk = ps_main.tile([HS, 512], F32, tag="ps")
                    for cc in range(NCC):
                        nc.tensor.matmul(
                            psq, lhsT=wq_sb[:, hh, ts(cc, HS)],
                            rhs=xnT[:, cc, ts(tq, 512)],
                            start=(cc == 0), stop=(cc == NCC - 1))
                    nc.scalar.activation(out=qT[:, hh, ts(tq, 512)], in_=psq,
                                         func=AF.Identity, bias=bq_sb[:, hh:hh + 1])
                    for cc in range(NCC):
                        nc.tensor.matmul(
                            psk, lhsT=wk_sb[:, hh, ts(cc, HS)],
                            rhs=xnT[:, cc, ts(tq, 512)],
                            start=(cc == 0), stop=(cc == NCC - 1))
                    nc.scalar.activation(out=kT[:, hh, ts(tq, 512)], in_=psk,
                                         func=AF.Identity, bias=bk_sb[:, hh:hh + 1])
                for st in range(NT):
                    psv = ps_sm.tile([P, HS], F32, tag="pacc")
                    for cc in range(NCC):
                        nc.tensor.matmul(
                            psv, lhsT=xnT[:, cc, ts(st, P)],
                            rhs=wv_sb[:, hh, ts(cc, HS)],
                            start=(cc == 0), stop=False)
                    # += bv (broadcast over tokens via rank-1 matmul)
                    nc.tensor.matmul(psv, lhsT=ones_row, rhs=bv_sb[:, ts(hh, HS)],
                                     start=False, stop=True)
                    nc.any.tensor_copy(out=vS[:, hh, st, :], in_=psv)

            # ================= Phase C: attention =================
            oT = [biga.tile([HS, T], BF16, name=f"oT{hh}", tag=f"oT{hh}") for hh in range(PAIRS)]
            for hh in range(PAIRS):
                recips = spool.tile([P, NT], F32, tag="recips")
                for i in range(T // 512):      # t quads (for E^T / AV)
                    ets = []
                    for j in range(4 * i + 4):  # s chunks of 128
                        pse = ps_main.tile([P, 512], F32, tag="ps")
                        nc.tensor.matmul(pse, lhsT=qT[:, hh, ts(j, P)],
                                         rhs=kT[:, hh, ts(i, 512)],
                                         start=True, stop=True)
                        et = etp.tile([P, 512], BF16, tag="et")
                        nc.scalar.activation(out=et, in_=pse, func=AF.Exp, scale=SCALE)
                        if j >= 4 * i:
                            u = j - 4 * i
                            if u > 0:
                                nc.gpsimd.memset(et[:, 0:u * P], 0.0)
                            # diag block: keep t >= s
                            nc.vector.tensor_mul(out=et[:, ts(u, P)],
                                                 in0=et[:, ts(u, P)], in1=triu_m)
                        ets.append(et)
                    for u in range(4):
                        r = 4 * i + u
                        L = P * (r + 1)
                        E = epool.tile([P, T], F32, tag="E")
                        sums = spool.tile([P, 8], F32, tag="sums")
                        nq = r // 4 + 1
                        for qd in range(nq):
                            W = 512 if qd < r // 4 else P * (r % 4 + 1)
                            pss = ps_main.tile([P, 512], F32, tag="ps")
                            nc.tensor.matmul(pss[:, :W], lhsT=kT[:, hh, ts(r, P)],
                                             rhs=qT[:, hh, qd * 512:qd * 512 + W],
                                             start=True, stop=True)
                            if qd < r // 4:
                                nc.scalar.activation(out=E[:, ts(qd, 512)], in_=pss,
                                                     func=AF.Exp, scale=SCALE,
                                                     accum_out=sums[:, qd:qd + 1])
                            else:
                                off = qd * 512
                                if W > P:
                                    nc.scalar.activation(
                                        out=E[:, off:off + W - P], in_=pss[:, :W - P],
                                        func=AF.Exp, scale=SCALE,
                                        accum_out=sums[:, qd:qd + 1])
                                else:
                                    nc.vector.memset(sums[:, qd:qd + 1], 0.0)
                                ds = off + W - P   # == P*r
                                nc.scalar.activation(out=E[:, ds:ds + P],
                                                     in_=pss[:, W - P:W],
                                                     func=AF.Exp, scale=SCALE)
                                nc.vector.tensor_mul(out=E[:, ds:ds + P],
                                                     in0=E[:, ds:ds + P], in1=tril_m)
                                nc.vector.reduce_sum(out=sums[:, nq:nq + 1],
                                                     in_=E[:, ds:ds + P], axis=AX.X)
                        tot = spool.tile([P, 1], F32, tag="tot")
                        nc.vector.reduce_sum(out=tot, in_=sums[:, :nq + 1], axis=AX.X)
                        nc.vector.reciprocal(out=recips[:, r:r + 1], in_=tot)
                        Eo = epool.tile([P, T], F32, tag="Eo")
                        nc.scalar.activation(out=Eo[:, :L], in_=E[:, :L],
                                             func=AF.Copy,
                                             scale=recips[:, r:r + 1])
                        nc.sync.dma_start(out=attn[hh, ts(r, P), 0:L], in_=Eo[:, :L])
                        # o row-tile: accumulate over valid s chunks
                        pso = ps_sm.tile([P, HS], F32, tag="pacc")
                        for j in range(r + 1):
                            nc.tensor.matmul(pso, lhsT=ets[j][:, ts(u, P)],
                                             rhs=vS[:, hh, j, :],
                                             start=(j == 0), stop=(j == r))
                        osb = opool.tile([P, HS], BF16, tag="osb")
                        nc.vector.tensor_scalar_mul(out=osb, in0=pso,
                                                    scalar1=recips[:, r:r + 1])
                        tp = ps_t.tile([P, P], BF16, tag="pst")
                        nc.tensor.transpose(out=tp[:HS, :], in_=osb, identity=ident)
                        nc.any.tensor_copy(out=oT[hh][:, ts(r, P)], in_=tp[:HS, :])

            # ================= Phase D: partial projection + RS =================
            for tt in range(NT):
                pp = ppool.tile([P, D], BF16, tag="pp")
                for half in range(2):
                    psp = ps_main.tile([P, 384], F32, tag="ps")
                    for hh in range(PAIRS):
                        nc.tensor.matmul(psp, lhsT=oT[hh][:, ts(tt, P)],
                                         rhs=wp_sb[:, hh, ts(half, 384)],
                                         start=(hh == 0), stop=(hh == PAIRS - 1))
                    nc.scalar.activation(out=pp[:, ts(half, 384)], in_=psp, func=AF.Copy)
                nc.sync.dma_start(
                    out=rs_in[tt // 4, (tt % 4) * P:(tt % 4 + 1) * P, :], in_=pp)

        nc.gpsimd.collective_compute(
            "ReduceScatter", ALU.add,
            replica_groups=[[0, 1, 2, 3], [4, 5, 6, 7]],
            ins=[rs_in.opt()], outs=[rs_out.opt()])

        # ================= Phase E: residual + LN2 + FFN =================
        with tc.tile_pool(name="bigb", bufs=1) as bigb:
            x2 = bigb.tile([P, 4, D], F32)
            xn2T = bigb.tile([P, NCC, TSL], BF16)
            for t2 in range(4):
                rso = ypool.tile([P, D], BF16, tag="rso")
                nc.gpsimd.dma_start(out=rso, in_=rs_out[ts(t2, P), :])
                xst = xstream.tile([P, D], F32, tag="xt")
                nc.gpsimd.dma_start(out=xst, in_=xsp[ts(t2, P), :])
                nc.vector.tensor_add(out=x2[:, t2, :], in0=xst, in1=rso)
                xn2 = xstream.tile([P, D], BF16, tag="xn")
                _layernorm(nc, spool, x2[:, t2, :], xn2, eps_t)
                for cc in range(NCC):
                    tp = ps_t.tile([P, P], BF16, tag="pst")
                    nc.tensor.transpose(out=tp, in_=xn2[:, ts(cc, P)], identity=ident)
                    nc.any.tensor_copy(out=xn2T[:, cc, ts(t2, P)], in_=tp)

            h1T = bigb.tile([P, NHT, TSL], BF16)
            w2_sb = bigb.tile([P, NHT, D], BF16)
            for ht in range(NHT):
                nc.gpsimd.dma_start(out=w2_sb[:, ht, :], in_=w2[ht])
                w1t = wstream.tile([P, D], BF16, tag="w1t")
                nc.gpsimd.dma_start(out=w1t, in_=w1[ht])
                psh = ps_main.tile([P, TSL], F32, tag="ps")
                for cc in range(NCC):
                    nc.tensor.matmul(psh, lhsT=w1t[:, ts(cc, P)], rhs=xn2T[:, cc, :],
                                     start=(cc == 0), stop=(cc == NCC - 1))
                nc.scalar.activation(out=h1T[:, ht, :], in_=psh, func=AF.Relu,
                                     bias=b1_sb[:, ht:ht + 1])
            for t2 in range(4):
                ysb = ypool.tile([P, D], F32, tag="ysb")
                for half in range(2):
                    ps2 = ps_main.tile([P, 384], F32, tag="ps")
                    for ht in range(NHT):
                        nc.tensor.matmul(ps2, lhsT=h1T[:, ht, ts(t2, P)],
                                         rhs=w2_sb[:, ht, ts(half, 384)],
                                         start=(ht == 0), stop=(ht == NHT - 1))
                    nc.vector.tensor_add(out=ysb[:, ts(half, 384)], in0=ps2,
                                         in1=x2[:, t2, ts(half, 384)])
                nc.vector.tensor_add(out=ysb, in0=ysb, in1=b2bc)
                nc.sync.dma_start(out=y[ts(t2, P), :], in_=ysb)

    nc.compile()
    return nc


def _prep_in_maps(inputs):
    x = np.asarray(inputs["x"], np.float32)
    Wq = np.asarray(inputs["Wq"], np.float32)
    Wk = np.asarray(inputs["Wk"], np.float32)
    Wv = np.asarray(inputs["Wv"], np.float32)
    bq = np.asarray(inputs["bq"], np.float32)
    bk = np.asarray(inputs["bk"], np.float32)
    bv = np.asarray(inputs["bv"], np.float32)
    Wp = np.asarray(inputs["Wp"], np.float32)
    bp = np.asarray(inputs["bp"], np.float32)
    W1 = np.asarray(inputs["W1"], np.float32)
    b1 = np.asarray(inputs["b1"], np.float32)
    W2 = np.asarray(inputs["W2"], np.float32)
    b2 = np.asarray(inputs["b2"], np.float32)

    def wqkv_slot(Wm, h):
        # [128, 6*64]: [p, cc*64+j] = Wm[h][j, cc*128+p]
        return np.ascontiguousarray(
            Wm[h].reshape(HS, NCC, P).transpose(2, 1, 0).reshape(P, NCC * HS)
        ).astype(BF)

    w1_host = np.ascontiguousarray(
        W1.reshape(NHT, P, NCC, P).transpose(0, 3, 2, 1).reshape(NHT, P, D)
    ).astype(BF)
    b1_host = np.ascontiguousarray(b1.reshape(NHT, P).T).astype(np.float32)
    w2_host = np.ascontiguousarray(
        W2.reshape(D, NHT, P).transpose(1, 2, 0)
    ).astype(BF)
    b2_host = b2.reshape(1, D).astype(np.float32)

    in_maps = []
    for c in range(NCORES):
        b, m = c // 4, c % 4
        hs = [3 * c % 12 + i for i in range(PAIRS)]
        im = {
            "xb": np.ascontiguousarray(x[b]),
            "xsp": np.ascontiguousarray(x[b, 512 * m:512 * (m + 1)] + bp[None, :]),
            "wq": np.stack([wqkv_slot(Wq, h) for h in hs]),
            "wk": np.stack([wqkv_slot(Wk, h) for h in hs]),
            "wv": np.stack([wqkv_slot(Wv, h) for h in hs]),
            "bq": np.ascontiguousarray(np.stack([bq[h] for h in hs], axis=1)),
            "bk": np.ascontiguousarray(np.stack([bk[h] for h in hs], axis=1)),
            "bv": np.concatenate([bv[h] for h in hs]).reshape(1, PAIRS * HS),
            "wp": np.stack(
                [np.ascontiguousarray(Wp[:, h * HS:(h + 1) * HS].T).astype(BF)
                 for h in hs]),
            "w1": w1_host,
            "b1": b1_host,
            "w2": w2_host,
            "b2": b2_host,
        }
        in_maps.append(im)
    return in_maps


_CACHED = {}


def run(inputs, trace=False):
    if "nc" not in _CACHED:
        _CACHED["nc"] = _build_program()
    nc = _CACHED["nc"]
    in_maps = _prep_in_maps(inputs)
    res = run_bass_kernel_spmd(nc, in_maps, list(range(NCORES)), trace=trace)
    ys = np.stack([np.asarray(res.results[c]["y"]) for c in range(NCORES)])
    attns = np.stack([np.asarray(res.results[c]["attn"]) for c in range(NCORES)])
    y_full = ys.reshape(2, T, D)
    attn_full = attns.reshape(2, H, T, T)
    return (y_full, attn_full), res


def kernel(**inputs):
    out, _ = run(inputs, trace=False)
    return out
